# revision 1
# baseline (speedup 1.0000x reference)
"""Trainium2 Bass kernel for nn_CausalSelfAttention (GQA + RoPE + qk-RMSNorm).

Strategy (Megatron-style head parallelism over 8 NeuronCores):
  - Each core owns 2 of the 16 q heads and the matching 1 of 8 kv heads.
  - Per core: QKV projection for its 512 rows of w_attn, RoPE + qk RMS norm,
    causal flash-style attention for its (2 q heads x 2 batches), and a
    partial output projection through its 256 columns of w_proj.
  - Host sums the 8 partial outputs (no on-device collectives).

All tensors are fed to the device pre-swizzled into SBUF-ready
[128, free...] layouts (bf16 for matmul operands).  Matmuls run in bf16 with
fp32 PSUM accumulation; softmax/statistics run in fp32.

Self-contained: hardcodes all shapes from the problem spec.
"""

import math
import numpy as np
import ml_dtypes
from contextlib import ExitStack

# ---- problem constants (hardcoded per spec) ----
B, T, C = 2, 2048, 2048
N_HEAD, N_KV_HEAD, HD = 16, 8, 128
KV_DIM = N_KV_HEAD * HD
EPS = 1.1920929e-07
N_CORES = 8
QH_PER_CORE = N_HEAD // N_CORES          # 2
TOK = B * T                              # 4096
P = 128
TG = 512                                 # token group (matmul N)
NT = TOK // TG                           # 8 token groups
KT = C // P                              # 16 contraction tiles
NGB = T // TG                            # 4 q groups per batch
NJB = T // P                             # 16 k tiles per batch
SCALE = 1.0 / math.sqrt(HD)

BF16 = ml_dtypes.bfloat16

_CACHE = {}


# --------------------------------------------------------------------------
# device program
# --------------------------------------------------------------------------

def _emit(tc, out_ap, t_in):
    import concourse.bass as bass  # noqa: F401
    import concourse.mybir as mybir

    f32 = mybir.dt.float32
    bf16 = mybir.dt.bfloat16
    AF = mybir.ActivationFunctionType
    nc = tc.nc

    x_d = t_in["x_sw"]
    wq_d = t_in["wq_sw"]
    wp_d = t_in["wp_sw"]
    cs_d = t_in["cs_sw"]
    mask_d = t_in["mask_sw"]
    eye_d = t_in["eye_sw"]
    ones_d = t_in["ones_sw"]

    with ExitStack() as root:
        const = root.enter_context(tc.tile_pool(name="const", bufs=1))
        # first QKV matmuls need only wq k0-3 + the first x half: issue those
        # DMAs first so PE starts ~10us earlier; bulk consts follow.
        wq_sb = const.tile([P, KT, TG], bf16)
        nc.sync.dma_start(out=wq_sb[:, 0:4, :], in_=wq_d[:, 0:4, :])
        x0_sb = const.tile([P, KT, TG], bf16, tag="x0")
        nc.sync.dma_start(out=x0_sb[:, 0:4, :], in_=x_d[:, 0, 0:4, :])
        nc.sync.dma_start(out=wq_sb[:, 4:16, :], in_=wq_d[:, 4:16, :])
        nc.sync.dma_start(out=x0_sb[:, 4:16, :], in_=x_d[:, 0, 4:16, :])
        ones_sb = const.tile([P, 1], bf16)
        nc.sync.dma_start(out=ones_sb[:], in_=ones_d)
        eye_sb = const.tile([P, P], bf16)
        nc.sync.dma_start(out=eye_sb[:], in_=eye_d)
        cs_sb = const.tile([P, 2, T], bf16)
        nc.sync.dma_start(out=cs_sb[:], in_=cs_d)
        mask_sb = const.tile([P, 4, TG], bf16)
        nc.sync.dma_start(out=mask_sb[:], in_=mask_d)
        wp_sb = const.tile([P, QH_PER_CORE, C], bf16)
        nc.sync.dma_start(out=wp_sb[:], in_=wp_d)
        eps_sb = const.tile([P, 1], f32)
        nc.vector.memset(eps_sb[:], EPS)
        onesm_sb = const.tile([P, P], bf16)
        nc.vector.memset(onesm_sb[:], 1.0)

        big = root.enter_context(tc.tile_pool(name="big", bufs=1))
        # post-rope, post-norm q (2 heads) and k, in [d, tok] layout
        qn = [big.tile([P, TOK], bf16, name=f"qn{m}", tag=f"qn{m}") for m in range(3)]
        v_sb = big.tile([P, TOK], bf16, tag="v")
        vT_sb = big.tile([P, 2 * NJB, P], bf16, tag="vT")   # [ktok, (b,j), d]
        yT = [big.tile([P, TOK], bf16, name=f"yT{h}", tag=f"yT{h}") for h in range(QH_PER_CORE)]

        # ------- stage 1+2: QKV projection + rope/norm + v transpose -------
        with ExitStack() as s1:
            xin = s1.enter_context(tc.tile_pool(name="xin", bufs=2))
            qkv_ps = s1.enter_context(tc.tile_pool(name="qkvps", bufs=3, space="PSUM"))
            ssq_ps = s1.enter_context(tc.tile_pool(name="ssqps", bufs=2, space="PSUM"))
            vt_ps = s1.enter_context(tc.tile_pool(name="vtps", bufs=2, space="PSUM"))
            sqp = s1.enter_context(tc.tile_pool(name="sq", bufs=3))
            srp = s1.enter_context(tc.tile_pool(name="sr", bufs=3))
            ropet = s1.enter_context(tc.tile_pool(name="ropet", bufs=2))

            for b in range(B):
                for nn in range(NT // B):
                    n = b * (NT // B) + nn
                    if n == 0:
                        xb = x0_sb
                    else:
                        xb = xin.tile([P, KT, TG], bf16)
                        nc.sync.dma_start(out=xb[:, 0:8, :], in_=x_d[:, n, 0:8, :])
                        nc.sync.dma_start(out=xb[:, 8:16, :], in_=x_d[:, n, 8:16, :])
                    for m in range(4):
                        ps = qkv_ps.tile([P, TG], f32)
                        for k in range(KT):
                            nc.tensor.matmul(
                                ps[:],
                                wq_sb[:, k, m * P:(m + 1) * P],
                                xb[:, k],
                                start=(k == 0),
                                stop=(k == KT - 1),
                            )
                        if m == 3:
                            nc.vector.tensor_copy(v_sb[:, n * TG:(n + 1) * TG], ps[:])
                        else:
                            # rms-norm: broadcast sum-of-squares via all-ones MM
                            sq = sqp.tile([P, TG], bf16)
                            nc.scalar.activation(sq[:], ps[:], AF.Square)
                            ssqb = ssq_ps.tile([P, TG], f32)
                            nc.tensor.matmul(
                                ssqb[:], onesm_sb[:], sq[:], start=True, stop=True
                            )
                            srb = srp.tile([P, TG], f32)
                            nc.scalar.activation(
                                srb[:], ssqb[:], AF.Sqrt,
                                bias=eps_sb[:], scale=1.0 / HD,
                            )
                            nc.vector.reciprocal_approx_fast(srb[:], srb[:])
                            # normalized copy psum -> sbuf (rope comes after;
                            # rotation commutes with the per-token scale)
                            nc.vector.tensor_mul(
                                qn[m][:, n * TG:(n + 1) * TG], ps[:], srb[:]
                            )
                # rope for this batch (k first: attention needs it earliest)
                sl = slice(b * T, (b + 1) * T)
                for m in (2, 0, 1):
                    t1 = ropet.tile([P, T], bf16, tag="t1")
                    xsw = ropet.tile([P, T], bf16, tag="xsw")
                    nc.gpsimd.dma_start(out=xsw[0:64, :], in_=qn[m][64:128, sl])
                    nc.gpsimd.dma_start(out=xsw[64:128, :], in_=qn[m][0:64, sl])
                    nc.vector.tensor_mul(t1[:], qn[m][:, sl], cs_sb[:, 0])
                    # t1 = [x1*c ; x2*c]; xsw*s2n = [x2*s ; -x1*s]
                    nc.vector.tensor_mul(xsw[:], xsw[:], cs_sb[:, 1])
                    nc.vector.tensor_add(qn[m][:, sl], t1[:], xsw[:])
                # v transposes for this batch
                for blk in range(b * NJB, (b + 1) * NJB):
                    tp = vt_ps.tile([P, P], bf16)
                    nc.tensor.transpose(
                        tp[:], v_sb[:, blk * P:(blk + 1) * P], eye_sb[:]
                    )
                    nc.vector.tensor_copy(vT_sb[:, blk], tp[:])

        # ---------------- stage 3: attention ------------------------------
        with ExitStack() as s3:
            s_ps = s3.enter_context(tc.tile_pool(name="sps", bufs=2, space="PSUM"))
            y_ps = s3.enter_context(tc.tile_pool(name="yps", bufs=2, space="PSUM"))
            d_ps = s3.enter_context(tc.tile_pool(name="dps", bufs=2, space="PSUM"))
            ptp = s3.enter_context(tc.tile_pool(name="pt", bufs=10))
            denp = s3.enter_context(tc.tile_pool(name="den", bufs=2))

            for b in range(B):
                for qh in range(QH_PER_CORE):
                    q_t, k_t = qn[qh], qn[2]
                    for g in range(NGB):
                        qsl = slice(b * T + g * TG, b * T + (g + 1) * TG)
                        jmax = 4 * g + 3
                        pts = []
                        for pr in range((jmax + 1) // 2):
                            sp2 = s_ps.tile([P, 2, TG], f32)
                            for jj in (0, 1):
                                j = 2 * pr + jj
                                off = (j - 4 * g) * P if j >= 4 * g else 0
                                nc.tensor.matmul(
                                    sp2[:, jj, off:],
                                    k_t[:, b * T + j * P: b * T + (j + 1) * P],
                                    q_t[:, qsl][:, off:],
                                    start=True,
                                    stop=True,
                                )
                            pt2 = ptp.tile([P, 2, TG], bf16)
                            if 2 * pr >= 4 * g:
                                # diagonal pair: per-j exp on the written range
                                for jj in (0, 1):
                                    j = 2 * pr + jj
                                    off = (j - 4 * g) * P
                                    nc.scalar.activation(
                                        pt2[:, jj, off:], sp2[:, jj, off:],
                                        AF.Exp, scale=SCALE,
                                    )
                                    if off:
                                        nc.gpsimd.memset(pt2[:, jj, 0:off], 0.0)
                                    nc.vector.tensor_mul(
                                        pt2[:, jj, off:off + P],
                                        pt2[:, jj, off:off + P],
                                        mask_sb[:, 0, 0:P],
                                    )
                                    pts.append(pt2[:, jj, :])
                            else:
                                nc.scalar.activation(pt2[:], sp2[:], AF.Exp,
                                                     scale=SCALE)
                                pts.append(pt2[:, 0, :])
                                pts.append(pt2[:, 1, :])
                        yp = y_ps.tile([P, TG], f32)
                        for j, pt in enumerate(pts):
                            nc.tensor.matmul(
                                yp[:], vT_sb[:, b * NJB + j], pt[:],
                                start=(j == 0), stop=(j == jmax),
                            )
                        # denominator, broadcast to all partitions via ones-mat
                        dp = d_ps.tile([P, TG], f32)
                        for j, pt in enumerate(pts):
                            nc.tensor.matmul(
                                dp[:], onesm_sb[:], pt[:],
                                start=(j == 0), stop=(j == jmax),
                            )
                        den = denp.tile([P, TG], f32)
                        nc.vector.reciprocal_approx_fast(den[:], dp[:])
                        nc.vector.tensor_mul(yT[qh][:, qsl], yp[:], den[:])

        # ---------------- stage 4: output projection ----------------------
        with ExitStack() as s4:
            o_ps = s4.enter_context(tc.tile_pool(name="ops", bufs=4, space="PSUM"))
            ostgp = s4.enter_context(tc.tile_pool(name="ostg", bufs=3))
            for tt in range(TOK // P):
                ost = ostgp.tile([P, C], bf16)
                for og in range(C // TG):
                    op = o_ps.tile([P, TG], f32)
                    nc.tensor.matmul(
                        op[:], yT[0][:, tt * P:(tt + 1) * P],
                        wp_sb[:, 0, og * TG:(og + 1) * TG],
                        start=True, stop=False,
                    )
                    nc.tensor.matmul(
                        op[:], yT[1][:, tt * P:(tt + 1) * P],
                        wp_sb[:, 1, og * TG:(og + 1) * TG],
                        start=False, stop=True,
                    )
                    if og % 2 == 0:
                        nc.vector.tensor_copy(ost[:, og * TG:(og + 1) * TG], op[:])
                    else:
                        nc.scalar.copy(ost[:, og * TG:(og + 1) * TG], op[:])
                nc.sync.dma_start(out=out_ap[tt * P:(tt + 1) * P, :], in_=ost[:])


def build_nc():
    """Build and compile the (single, shared across cores) Bass program."""
    if "nc" in _CACHE:
        return _CACHE["nc"]
    import concourse.mybir as mybir
    import concourse.tile as tile
    from concourse import bacc

    f32 = mybir.dt.float32  # noqa: F841
    bf16 = mybir.dt.bfloat16

    nc = bacc.Bacc("TRN2", target_bir_lowering=False, debug=False)
    shapes = {
        "x_sw": ((P, NT, KT, TG), bf16),
        "wq_sw": ((P, KT, TG), bf16),
        "wp_sw": ((P, QH_PER_CORE, C), bf16),
        "cs_sw": ((P, 2, T), bf16),
        "mask_sw": ((P, 4, TG), bf16),
        "eye_sw": ((P, P), bf16),
        "ones_sw": ((P, 1), bf16),
    }
    t_in = {
        name: nc.dram_tensor(name, shape, dt, kind="ExternalInput").ap()
        for name, (shape, dt) in shapes.items()
    }
    out_ap = nc.dram_tensor("out", (TOK, C), bf16, kind="ExternalOutput").ap()

    with tile.TileContext(nc) as tc:
        _emit(tc, out_ap, t_in)
    nc.compile()
    _CACHE["nc"] = nc
    return nc


# --------------------------------------------------------------------------
# host-side data preparation
# --------------------------------------------------------------------------

def _swizzle_ktiles(a2d):
    """[R*128, F] -> [128, R, F] picking partition-within-tile as leading."""
    r128, f = a2d.shape
    r = r128 // P
    return np.ascontiguousarray(a2d.reshape(r, P, f).transpose(1, 0, 2))


def host_prep(x, w_attn, w_proj, cos, sin):
    x = np.asarray(x, np.float32)
    w_attn = np.asarray(w_attn, np.float32)
    w_proj = np.asarray(w_proj, np.float32)
    cos = np.asarray(cos, np.float32).reshape(T, HD // 2)
    sin = np.asarray(sin, np.float32).reshape(T, HD // 2)

    # x: (B,T,C) -> xT (C, TOK) -> [128, n, k, t]
    xT = x.reshape(TOK, C).T                        # (C, TOK)
    x_sw = (
        xT.reshape(KT, P, NT, TG).transpose(1, 2, 0, 3)  # (P, n, k, t)
    )
    x_sw = np.ascontiguousarray(x_sw).astype(BF16)

    # cos/sin duplicated across both 64-partition halves: [128, 2, T]
    c2 = np.concatenate([cos.T, cos.T], axis=0)     # (128, T)
    s2 = np.concatenate([sin.T, -sin.T], axis=0)    # sign-folded for rope add
    cs_sw = np.stack([c2, s2], axis=1).astype(BF16)  # (128, 2, T)

    # causal masks for the 4 diagonal offsets: keep col >= row + 128*off
    col = np.arange(TG)[None, :]
    row = np.arange(P)[:, None]
    mask_sw = np.stack(
        [(col >= row + P * off) for off in range(4)], axis=1
    ).astype(BF16)                                   # (128, 4, 512)

    eye_sw = np.eye(P, dtype=np.float32).astype(BF16)
    ones_sw = np.ones((P, 1), np.float32).astype(BF16)

    in_maps = []
    for c in range(N_CORES):
        qrows = w_attn[QH_PER_CORE * HD * c: QH_PER_CORE * HD * (c + 1)]
        krows = w_attn[C + HD * c: C + HD * (c + 1)]
        vrows = w_attn[C + KV_DIM + HD * c: C + KV_DIM + HD * (c + 1)]
        w_sel = np.concatenate([qrows, krows, vrows], axis=0)   # (512, C)
        wq_sw = _swizzle_ktiles(w_sel.T).astype(BF16)           # (128, 16, 512)

        wp_sel = w_proj[:, QH_PER_CORE * HD * c: QH_PER_CORE * HD * (c + 1)]
        wp_sw = _swizzle_ktiles(np.ascontiguousarray(wp_sel.T)).astype(BF16)

        in_maps.append({
            "x_sw": x_sw,
            "wq_sw": np.ascontiguousarray(wq_sw.reshape(P, KT, TG)),
            "wp_sw": np.ascontiguousarray(wp_sw.reshape(P, QH_PER_CORE, C)),
            "cs_sw": cs_sw,
            "mask_sw": mask_sw,
            "eye_sw": eye_sw,
            "ones_sw": ones_sw,
        })
    return in_maps


def run_on_hw(in_maps, trace=False, **kwargs):
    from concourse import bass_utils

    nc = build_nc()
    return bass_utils.run_bass_kernel_spmd(
        nc, in_maps, core_ids=list(range(N_CORES)), trace=trace, **kwargs
    )


def kernel(x, w_attn, w_proj, cos, sin):
    in_maps = host_prep(x, w_attn, w_proj, cos, sin)
    res = run_on_hw(in_maps)
    out = np.zeros((TOK, C), np.float64)
    for r in res.results:
        out += r["out"].astype(np.float64)
    return out.astype(np.float32).reshape(B, T, C)



# revision 2
# speedup vs baseline: 1.0687x; 1.0687x over previous
"""Trainium2 Bass kernel for nn_CausalSelfAttention (GQA + RoPE + qk-RMSNorm).

Sharding: batch x head-quad over 8 NeuronCores.
  - Core c: batch = c // 4, quad = c % 4.
  - Each core owns 4 of the 16 q heads (4*quad .. 4*quad+3) and the matching
    2 of 8 kv heads (2*quad, 2*quad+1) for ONE batch element.
  - Per core: QKV projection for its 1024 rows of w_attn over its batch's
    2048 tokens, RoPE + qk RMS norm, causal attention, partial output
    projection through its 512 columns of w_proj.
  - Host sums the 4 partial outputs per batch (no on-device collectives).

Fused per-token-group pipeline: for each 512-token group g we run
QKV -> rope/norm -> attention (flash-style, causal-sliced) -> out-proj, so
the tensor engine always has dense matmul work while exp/softmax runs on
the scalar/vector engines.

Matmuls run in bf16 with fp32 PSUM accumulation; softmax/statistics fp32.
Self-contained: hardcodes all shapes from the problem spec.
"""

import math
import numpy as np
import ml_dtypes
from contextlib import ExitStack

# ---- problem constants (hardcoded per spec) ----
B, T, C = 2, 2048, 2048
N_HEAD, N_KV_HEAD, HD = 16, 8, 128
KV_DIM = N_KV_HEAD * HD
EPS = 1.1920929e-07
N_CORES = 8
P = 128
TG = 512                                 # token group (matmul N)
G = T // TG                              # 4 token groups per core
KT = C // P                              # 16 contraction tiles
QH = 4                                   # q heads per core
KVH = 2                                  # kv heads per core
MQ = QH + 2 * KVH                        # 8 row-quarters of the 1024 QKV rows
NJ = T // P                              # 16 k tiles
SCALE = 1.0 / math.sqrt(HD)

BF16 = ml_dtypes.bfloat16

_CACHE = {}


# --------------------------------------------------------------------------
# device program
# --------------------------------------------------------------------------

def _emit(tc, out_ap, t_in):
    import concourse.bass as bass  # noqa: F401
    import concourse.mybir as mybir

    f32 = mybir.dt.float32
    bf16 = mybir.dt.bfloat16
    AF = mybir.ActivationFunctionType
    nc = tc.nc

    x_d = t_in["x_sw"]
    wq_d = t_in["wq_sw"]
    wp_d = t_in["wp_sw"]
    cs_d = t_in["cs_sw"]
    trineg_d = t_in["trineg_sw"]
    eye_d = t_in["eye_sw"]

    with ExitStack() as root:
        const = root.enter_context(tc.tile_pool(name="const", bufs=1))
        xin = root.enter_context(tc.tile_pool(name="xin", bufs=2))
        # fine-grained interleaved staging: QKV(g=0) runs k-outer over the
        # q-head half of wq, so chunk k-tiles of wq/x land just ahead of use.
        # wq on the sync queue, x on the gpsimd queue -- parallel streams.
        wq_sb = const.tile([P, KT, MQ * P], bf16)
        x0_sb = xin.tile([P, KT, TG], bf16, tag="xb")
        for k0 in range(0, KT, 2):
            nc.sync.dma_start(out=wq_sb[:, k0:k0 + 2, 0:4 * P],
                              in_=wq_d[:, k0:k0 + 2, 0:4 * P])
            nc.sync.dma_start(out=x0_sb[:, k0:k0 + 2, :],
                              in_=x_d[:, 0, k0:k0 + 2, :])
        for k0 in range(0, KT, 4):
            nc.sync.dma_start(out=wq_sb[:, k0:k0 + 4, 4 * P:],
                              in_=wq_d[:, k0:k0 + 4, 4 * P:])
        eye_sb = const.tile([P, P], bf16)
        nc.sync.dma_start(out=eye_sb[:], in_=eye_d)
        cs_sb = const.tile([P, 2, T], bf16)
        nc.sync.dma_start(out=cs_sb[:], in_=cs_d)
        trineg_sb = const.tile([P, P], bf16)
        nc.sync.dma_start(out=trineg_sb[:], in_=trineg_d)
        wp_sb = const.tile([P, QH, C], bf16)
        nc.sync.dma_start(out=wp_sb[:], in_=wp_d)
        eps_sb = const.tile([P, 1], f32)
        nc.vector.memset(eps_sb[:], EPS)
        onesm_sb = const.tile([P, P], bf16)
        nc.vector.memset(onesm_sb[:], 1.0)

        big = root.enter_context(tc.tile_pool(name="big", bufs=1))
        # post-rope, post-norm q (4 heads) + k (2 heads), [d, tok] layout
        qn = [big.tile([P, T], bf16, name=f"qn{m}", tag=f"qn{m}")
              for m in range(6)]
        vT_sb = big.tile([P, KVH, NJ, P], bf16, tag="vT")  # [ktok, vh, j, d]
        yT = [big.tile([P, T], bf16, name=f"yT{h}", tag=f"yT{h}")
              for h in range(QH)]

        mm_ps = root.enter_context(tc.tile_pool(name="mmps", bufs=2, space="PSUM"))
        s_ps = root.enter_context(tc.tile_pool(name="sps", bufs=2, space="PSUM"))
        y_ps = root.enter_context(tc.tile_pool(name="yps", bufs=2, space="PSUM"))
        d_ps = root.enter_context(tc.tile_pool(name="dps", bufs=2, space="PSUM"))
        sqp = root.enter_context(tc.tile_pool(name="sq", bufs=3))
        srp = root.enter_context(tc.tile_pool(name="sr", bufs=2))
        ptp = root.enter_context(tc.tile_pool(name="pt", bufs=8))
        pap = root.enter_context(tc.tile_pool(name="pa", bufs=4))
        denp = root.enter_context(tc.tile_pool(name="den", bufs=2))
        vtmp = root.enter_context(tc.tile_pool(name="vtmp", bufs=2))
        xswp = root.enter_context(tc.tile_pool(name="xswp", bufs=6))
        ropet = root.enter_context(tc.tile_pool(name="ropet", bufs=3))
        ostg = root.enter_context(tc.tile_pool(name="ost", bufs=2))

        def emit_qkv_rope(g):
            """QKV projection + norm + rope for token group g."""
            gsl = slice(g * TG, (g + 1) * TG)
            if g == 0:
                xb = x0_sb
            else:
                xb = xin.tile([P, KT, TG], bf16, tag="xb", name="xb")
                nc.sync.dma_start(out=xb[:, 0:8, :], in_=x_d[:, g, 0:8, :])
                nc.sync.dma_start(out=xb[:, 8:16, :], in_=x_d[:, g, 8:16, :])
            xsws = {}

            def qkv_post(m, ps, mi):
                if m < 6:
                    # free the PSUM slot immediately: copy to SBUF first,
                    # then the whole norm chain runs off the SBUF copy, so
                    # an ACT table switch can't back up the matmul pipeline
                    nc.vector.tensor_copy(qn[m][:, gsl], ps[:])
                    # rms-norm: broadcast sum-of-squares via all-ones MM
                    sq = sqp.tile([P, TG], bf16)
                    nc.scalar.activation(sq[:], qn[m][:, gsl], AF.Square)
                    ssq = s_ps.tile([P, TG], f32, tag="s")
                    nc.tensor.matmul(ssq[:], onesm_sb[:], sq[:],
                                     start=True, stop=True)
                    srb = srp.tile([P, TG], f32)
                    nc.scalar.activation(srb[:], ssq[:], AF.Sqrt,
                                         bias=eps_sb[:], scale=1.0 / HD)
                    nc.vector.reciprocal_approx_fast(srb[:], srb[:])
                    nc.vector.tensor_mul(qn[m][:, gsl], qn[m][:, gsl], srb[:])
                    # issue the rope half-swap immediately; consumed after
                    # the m-loop.  Alternate DMA queues to avoid serializing.
                    xsw = xswp.tile([P, TG], bf16, tag="xsw")
                    eng = nc.gpsimd if mi % 2 == 0 else nc.sync
                    eng.dma_start(out=xsw[0:64, :], in_=qn[m][64:128, gsl])
                    eng.dma_start(out=xsw[64:128, :], in_=qn[m][0:64, gsl])
                    xsws[m] = xsw
                else:
                    vh = m - 6
                    vtm = vtmp.tile([P, TG], bf16)
                    nc.vector.tensor_copy(vtm[:], ps[:])
                    for jj in range(4):
                        tp = s_ps.tile([P, P], bf16, tag="s")
                        nc.tensor.transpose(
                            tp[:], vtm[:, jj * P:(jj + 1) * P], eye_sb[:])
                        nc.vector.tensor_copy(vT_sb[:, vh, 4 * g + jj], tp[:])

            if g == 0:
                # k-outer in two 4-quarter batches (4 PSUM banks each) so the
                # PE consumes wq/x chunks as the staged DMAs land
                for batch in ((0, 1, 2, 3), (4, 5, 6, 7)):
                    pss = [mm_ps.tile([P, TG], f32, tag="mm", name="ps_a"),
                           mm_ps.tile([P, TG], f32, tag="mm", name="ps_b"),
                           y_ps.tile([P, TG], f32, tag="y", name="ps_c"),
                           d_ps.tile([P, TG], f32, tag="d", name="ps_d")]
                    for k in range(KT):
                        for i, m in enumerate(batch):
                            nc.tensor.matmul(
                                pss[i][:],
                                wq_sb[:, k, m * P:(m + 1) * P],
                                xb[:, k],
                                start=(k == 0),
                                stop=(k == KT - 1),
                            )
                    for i, m in enumerate(batch):
                        qkv_post(m, pss[i], i)
            else:
                for mi, m in enumerate((4, 5, 0, 1, 2, 3, 6, 7)):
                    ps = mm_ps.tile([P, TG], f32, tag="mm")
                    for k in range(KT):
                        nc.tensor.matmul(
                            ps[:],
                            wq_sb[:, k, m * P:(m + 1) * P],
                            xb[:, k],
                            start=(k == 0),
                            stop=(k == KT - 1),
                        )
                    qkv_post(m, ps, mi)

            # rope (k quarters first)
            for m in (4, 5, 0, 1, 2, 3):
                xsw = xsws[m]
                t1 = ropet.tile([P, TG], bf16, tag="t1")
                nc.vector.tensor_mul(t1[:], qn[m][:, gsl], cs_sb[:, 0, gsl])
                nc.vector.tensor_mul(xsw[:], xsw[:], cs_sb[:, 1, gsl])
                nc.vector.tensor_add(qn[m][:, gsl], t1[:], xsw[:])

        def emit_attn_outproj(g):
            """Attention + out-projection for token group g."""
            gsl = slice(g * TG, (g + 1) * TG)
            jn = 4 * g + 4
            for qh in range(QH):
                kv = qh // 2
                k_t = qn[4 + kv]
                q_g = qn[qh][:, gsl]
                yp = y_ps.tile([P, TG], f32, tag="y")
                dp = d_ps.tile([P, TG], f32, tag="d")
                pts = []   # (ap, off) pending for the den chain
                for j in range(jn):
                    off = (j - 4 * g) * P if j >= 4 * g else 0
                    diag = j >= 4 * g
                    s = s_ps.tile([P, TG], f32, tag="s")
                    nc.tensor.matmul(
                        s[:, off:],
                        k_t[:, j * P:(j + 1) * P],
                        q_g[:, off:],
                        start=True,
                        stop=not diag,
                        skip_group_check=diag,
                    )
                    if diag:
                        # add -1e30 to the below-diagonal triangle on PE, so
                        # exp maps it to 0 (no cross-engine mask dependency)
                        nc.tensor.matmul(
                            s[:, off:off + P], trineg_sb[:], eye_sb[:],
                            start=False, stop=True,
                            skip_group_check=True,
                        )
                    pt = ptp.tile([P, TG], bf16)
                    nc.scalar.activation(pt[:, off:], s[:, off:], AF.Exp,
                                         scale=SCALE)
                    nc.tensor.matmul(
                        yp[:, off:], vT_sb[:, kv, j], pt[:, off:],
                        start=(j == 0), stop=(j == jn - 1),
                        skip_group_check=True,
                    )
                    # denominator: pair-add full tiles on DVE, chain on PE
                    if off == 0 and j % 2 == 0 and j + 1 < 4 * g:
                        pts.append((pt, -1))    # -1: waiting for partner
                    elif off == 0 and pts and pts[-1][1] == -1:
                        pa = pap.tile([P, TG], bf16)
                        nc.vector.tensor_add(pa[:], pts[-1][0][:], pt[:])
                        pts[-1] = (pa, 0)
                    else:
                        pts.append((pt, off))
                nd = len(pts)
                for i, (pa, off) in enumerate(pts):
                    assert off >= 0
                    nc.tensor.matmul(
                        dp[:, off:], onesm_sb[:], pa[:, off:],
                        start=(i == 0), stop=(i == nd - 1),
                        skip_group_check=True,
                    )
                den = denp.tile([P, TG], f32)
                if qh == QH - 1:
                    # chunk recip+mul per token-tile: the last head gates the
                    # out-projection, so let its first tile finish early
                    for u in range(4):
                        usl = slice(u * P, (u + 1) * P)
                        nc.vector.reciprocal_approx_fast(den[:, usl],
                                                         dp[:, usl])
                        nc.vector.tensor_mul(
                            yT[qh][:, g * TG + u * P: g * TG + (u + 1) * P],
                            yp[:, usl], den[:, usl])
                else:
                    nc.vector.reciprocal_approx_fast(den[:], dp[:])
                    nc.vector.tensor_mul(yT[qh][:, gsl], yp[:], den[:])

            # out-projection for this group
            for tt in range(4 * g, 4 * g + 4):
                ost = ostg.tile([P, C], bf16)
                for og in range(4):
                    op = mm_ps.tile([P, TG], f32, tag="mm", name="op")
                    for h in range(QH):
                        nc.tensor.matmul(
                            op[:], yT[h][:, tt * P:(tt + 1) * P],
                            wp_sb[:, h, og * TG:(og + 1) * TG],
                            start=(h == 0), stop=(h == QH - 1),
                        )
                    if og % 2 == 0:
                        nc.vector.tensor_copy(ost[:, og * TG:(og + 1) * TG], op[:])
                    else:
                        nc.scalar.copy(ost[:, og * TG:(og + 1) * TG], op[:])
                nc.sync.dma_start(out=out_ap[tt * P:(tt + 1) * P, :], in_=ost[:])

        # software pipeline: emit QKV(g+1) BEFORE attention(g) so the
        # scheduler can fill exp-gated attention bubbles with QKV matmuls
        for i in range(G + 1):
            if i < G:
                emit_qkv_rope(i)
            if i >= 1:
                emit_attn_outproj(i - 1)


def build_nc():
    """Build and compile the (single, shared across cores) Bass program."""
    if "nc" in _CACHE:
        return _CACHE["nc"]
    import concourse.mybir as mybir
    import concourse.tile as tile
    from concourse import bacc

    bf16 = mybir.dt.bfloat16

    nc = bacc.Bacc("TRN2", target_bir_lowering=False, debug=False)
    shapes = {
        "x_sw": ((P, G, KT, TG), bf16),
        "wq_sw": ((P, KT, MQ * P), bf16),
        "wp_sw": ((P, QH, C), bf16),
        "cs_sw": ((P, 2, T), bf16),
        "trineg_sw": ((P, P), bf16),
        "eye_sw": ((P, P), bf16),
    }
    t_in = {
        name: nc.dram_tensor(name, shape, dt, kind="ExternalInput").ap()
        for name, (shape, dt) in shapes.items()
    }
    out_ap = nc.dram_tensor("out", (T, C), bf16, kind="ExternalOutput").ap()

    with tile.TileContext(nc) as tc:
        _emit(tc, out_ap, t_in)
    nc.compile()
    _CACHE["nc"] = nc
    return nc


# --------------------------------------------------------------------------
# host-side data preparation
# --------------------------------------------------------------------------

def _swizzle_ktiles(a2d):
    """[R*128, F] -> [128, R, F] picking partition-within-tile as leading."""
    r128, f = a2d.shape
    r = r128 // P
    return np.ascontiguousarray(a2d.reshape(r, P, f).transpose(1, 0, 2))


def host_prep(x, w_attn, w_proj, cos, sin):
    x = np.asarray(x, np.float32)
    w_attn = np.asarray(w_attn, np.float32)
    w_proj = np.asarray(w_proj, np.float32)
    cos = np.asarray(cos, np.float32).reshape(T, HD // 2)
    sin = np.asarray(sin, np.float32).reshape(T, HD // 2)

    # x per batch: (T, C) -> [128, g, k, t]
    x_sws = []
    for b in range(B):
        xb = x[b].reshape(G, TG, KT, P).transpose(3, 0, 2, 1)
        x_sws.append(np.ascontiguousarray(xb).astype(BF16))

    # cos/sin duplicated across both 64-partition halves: [128, 2, T]
    c2 = np.concatenate([cos.T, cos.T], axis=0)     # (128, T)
    s2 = np.concatenate([sin.T, -sin.T], axis=0)    # sign-folded for rope add
    cs_sw = np.stack([c2, s2], axis=1).astype(BF16)  # (128, 2, T)

    col = np.arange(P)[None, :]
    row = np.arange(P)[:, None]
    # M[r,c] = 0 where causal-live (c >= r), -1e30 where masked; the device
    # adds M to the diagonal score block via lhsT = M.T (out[i,j] = lhsT[j,i])
    m_mask = np.where(col >= row, 0.0, -1e30).astype(np.float32)
    trineg_sw = np.ascontiguousarray(m_mask.T).astype(BF16)
    eye_sw = np.eye(P, dtype=np.float32).astype(BF16)

    in_maps = []
    for c in range(N_CORES):
        b, q = divmod(c, 4)
        qrows = w_attn[QH * HD * q: QH * HD * (q + 1)]
        krows = w_attn[C + KVH * HD * q: C + KVH * HD * (q + 1)]
        vrows = w_attn[C + KV_DIM + KVH * HD * q: C + KV_DIM + KVH * HD * (q + 1)]
        w_sel = np.concatenate([qrows, krows, vrows], axis=0)   # (1024, C)
        wq_sw = _swizzle_ktiles(w_sel.T).astype(BF16)           # (128, 16, 1024)

        wp_sel = w_proj[:, QH * HD * q: QH * HD * (q + 1)]      # (C, 512)
        wp_sw = _swizzle_ktiles(np.ascontiguousarray(wp_sel.T)).astype(BF16)

        in_maps.append({
            "x_sw": x_sws[b],
            "wq_sw": np.ascontiguousarray(wq_sw),
            "wp_sw": np.ascontiguousarray(wp_sw),   # (128, 4, 2048)
            "cs_sw": cs_sw,
            "trineg_sw": trineg_sw,
            "eye_sw": eye_sw,
        })
    return in_maps


def run_on_hw(in_maps, trace=False, **kwargs):
    from concourse import bass_utils

    nc = build_nc()
    return bass_utils.run_bass_kernel_spmd(
        nc, in_maps, core_ids=list(range(N_CORES)), trace=trace, **kwargs
    )


def gather(res):
    """Sum the 4 partial outputs per batch -> (B, T, C) float32."""
    out = np.zeros((B, T, C), np.float32)
    for c, r in enumerate(res.results):
        out[c // 4] += r["out"].astype(np.float32)
    return out


def kernel(x, w_attn, w_proj, cos, sin):
    in_maps = host_prep(x, w_attn, w_proj, cos, sin)
    res = run_on_hw(in_maps)
    return gather(res)


# revision 3
# speedup vs baseline: 1.0710x; 1.0021x over previous
"""Trainium2 Bass kernel for nn_CausalSelfAttention (GQA + RoPE + qk-RMSNorm).

Sharding: batch x head-quad over 8 NeuronCores.
  - Core c: batch = c // 4, quad = c % 4.
  - Each core owns 4 of the 16 q heads (4*quad .. 4*quad+3) and the matching
    2 of 8 kv heads (2*quad, 2*quad+1) for ONE batch element.
  - Per core: QKV projection for its 1024 rows of w_attn over its batch's
    2048 tokens, RoPE + qk RMS norm, causal attention, partial output
    projection through its 512 columns of w_proj.
  - Host sums the 4 partial outputs per batch (no on-device collectives).

Fused per-token-group pipeline: for each 512-token group g we run
QKV -> rope/norm -> attention (flash-style, causal-sliced) -> out-proj, so
the tensor engine always has dense matmul work while exp/softmax runs on
the scalar/vector engines.

Matmuls run in bf16 with fp32 PSUM accumulation; softmax/statistics fp32.
Self-contained: hardcodes all shapes from the problem spec.
"""

import math
import numpy as np
import ml_dtypes
from contextlib import ExitStack

# ---- problem constants (hardcoded per spec) ----
B, T, C = 2, 2048, 2048
N_HEAD, N_KV_HEAD, HD = 16, 8, 128
KV_DIM = N_KV_HEAD * HD
EPS = 1.1920929e-07
N_CORES = 8
P = 128
TG = 512                                 # token group (matmul N)
G = T // TG                              # 4 token groups per core
KT = C // P                              # 16 contraction tiles
QH = 4                                   # q heads per core
KVH = 2                                  # kv heads per core
MQ = QH + 2 * KVH                        # 8 row-quarters of the 1024 QKV rows
NJ = T // P                              # 16 k tiles
SCALE = 1.0 / math.sqrt(HD)

BF16 = ml_dtypes.bfloat16

_CACHE = {}


# --------------------------------------------------------------------------
# device program
# --------------------------------------------------------------------------

def _emit(tc, out_ap, t_in):
    import concourse.bass as bass  # noqa: F401
    import concourse.mybir as mybir

    f32 = mybir.dt.float32
    bf16 = mybir.dt.bfloat16
    AF = mybir.ActivationFunctionType
    nc = tc.nc

    x_d = t_in["x_sw"]
    wq_d = t_in["wq_sw"]
    wp_d = t_in["wp_sw"]
    cs_d = t_in["cs_sw"]
    trineg_d = t_in["trineg_sw"]
    eye_d = t_in["eye_sw"]

    with ExitStack() as root:
        const = root.enter_context(tc.tile_pool(name="const", bufs=1))
        xin = root.enter_context(tc.tile_pool(name="xin", bufs=2))
        # fine-grained interleaved staging: QKV(g=0) runs k-outer over the
        # q-head half of wq, so chunk k-tiles of wq/x land just ahead of use.
        # wq on the sync queue, x on the gpsimd queue -- parallel streams.
        wq_sb = const.tile([P, KT, MQ * P], bf16)
        x0_sb = xin.tile([P, KT, TG], bf16, tag="xb")
        for k0 in range(0, KT, 2):
            nc.sync.dma_start(out=wq_sb[:, k0:k0 + 2, 0:4 * P],
                              in_=wq_d[:, k0:k0 + 2, 0:4 * P])
            nc.sync.dma_start(out=x0_sb[:, k0:k0 + 2, :],
                              in_=x_d[:, 0, k0:k0 + 2, :])
        for k0 in range(0, KT, 4):
            nc.sync.dma_start(out=wq_sb[:, k0:k0 + 4, 4 * P:],
                              in_=wq_d[:, k0:k0 + 4, 4 * P:])
        eye_sb = const.tile([P, P], bf16)
        nc.sync.dma_start(out=eye_sb[:], in_=eye_d)
        cs_sb = const.tile([P, 2, T], bf16)
        nc.sync.dma_start(out=cs_sb[:], in_=cs_d)
        trineg_sb = const.tile([P, P], bf16)
        nc.sync.dma_start(out=trineg_sb[:], in_=trineg_d)
        wp_sb = const.tile([P, QH, C], bf16)
        nc.sync.dma_start(out=wp_sb[:], in_=wp_d)
        eps_sb = const.tile([P, 1], f32)
        nc.vector.memset(eps_sb[:], EPS)
        onesm_sb = const.tile([P, P], bf16)
        nc.vector.memset(onesm_sb[:], 1.0)

        big = root.enter_context(tc.tile_pool(name="big", bufs=1))
        # post-rope, post-norm q (4 heads) + k (2 heads), [d, tok] layout
        qn = [big.tile([P, T], bf16, name=f"qn{m}", tag=f"qn{m}")
              for m in range(6)]
        vT_sb = big.tile([P, KVH, NJ, P], bf16, tag="vT")  # [ktok, vh, j, d]
        yT = [big.tile([P, T], bf16, name=f"yT{h}", tag=f"yT{h}")
              for h in range(QH)]

        mm_ps = root.enter_context(tc.tile_pool(name="mmps", bufs=2, space="PSUM"))
        s_ps = root.enter_context(tc.tile_pool(name="sps", bufs=3, space="PSUM"))
        y_ps = root.enter_context(tc.tile_pool(name="yps", bufs=2, space="PSUM"))
        d_ps = root.enter_context(tc.tile_pool(name="dps", bufs=1, space="PSUM"))
        sqp = root.enter_context(tc.tile_pool(name="sq", bufs=3))
        srp = root.enter_context(tc.tile_pool(name="sr", bufs=2))
        ptp = root.enter_context(tc.tile_pool(name="pt", bufs=8))
        pap = root.enter_context(tc.tile_pool(name="pa", bufs=4))
        denp = root.enter_context(tc.tile_pool(name="den", bufs=2))
        vtmp = root.enter_context(tc.tile_pool(name="vtmp", bufs=2))
        xswp = root.enter_context(tc.tile_pool(name="xswp", bufs=6))
        ropet = root.enter_context(tc.tile_pool(name="ropet", bufs=3))
        ostg = root.enter_context(tc.tile_pool(name="ost", bufs=2))

        def emit_qkv_rope(g):
            """QKV projection + norm + rope for token group g."""
            gsl = slice(g * TG, (g + 1) * TG)
            if g == 0:
                xb = x0_sb
            else:
                xb = xin.tile([P, KT, TG], bf16, tag="xb", name="xb")
                nc.sync.dma_start(out=xb[:, 0:8, :], in_=x_d[:, g, 0:8, :])
                nc.sync.dma_start(out=xb[:, 8:16, :], in_=x_d[:, g, 8:16, :])
            xsws = {}

            def qkv_post(m, ps, mi):
                if m < 6:
                    # free the PSUM slot immediately: copy to SBUF first,
                    # then the whole norm chain runs off the SBUF copy, so
                    # an ACT table switch can't back up the matmul pipeline
                    nc.vector.tensor_copy(qn[m][:, gsl], ps[:])
                    # rms-norm: broadcast sum-of-squares via all-ones MM
                    sq = sqp.tile([P, TG], bf16)
                    nc.scalar.activation(sq[:], qn[m][:, gsl], AF.Square)
                    ssq = s_ps.tile([P, TG], f32, tag="s")
                    nc.tensor.matmul(ssq[:], onesm_sb[:], sq[:],
                                     start=True, stop=True)
                    srb = srp.tile([P, TG], f32)
                    nc.scalar.activation(srb[:], ssq[:], AF.Sqrt,
                                         bias=eps_sb[:], scale=1.0 / HD)
                    nc.vector.reciprocal_approx_fast(srb[:], srb[:])
                    nc.vector.tensor_mul(qn[m][:, gsl], qn[m][:, gsl], srb[:])
                    # issue the rope half-swap immediately; consumed after
                    # the m-loop.  Alternate DMA queues to avoid serializing.
                    xsw = xswp.tile([P, TG], bf16, tag="xsw")
                    eng = nc.gpsimd if mi % 2 == 0 else nc.sync
                    eng.dma_start(out=xsw[0:64, :], in_=qn[m][64:128, gsl])
                    eng.dma_start(out=xsw[64:128, :], in_=qn[m][0:64, gsl])
                    xsws[m] = xsw
                else:
                    vh = m - 6
                    vtm = vtmp.tile([P, TG], bf16)
                    nc.vector.tensor_copy(vtm[:], ps[:])
                    for jj in range(4):
                        tp = s_ps.tile([P, P], bf16, tag="s")
                        nc.tensor.transpose(
                            tp[:], vtm[:, jj * P:(jj + 1) * P], eye_sb[:])
                        nc.vector.tensor_copy(vT_sb[:, vh, 4 * g + jj], tp[:])

            if g == 0:
                # k-outer in two 4-quarter batches (4 PSUM banks each) so the
                # PE consumes wq/x chunks as the staged DMAs land
                for batch in ((0, 1, 2, 3), (4, 5, 6, 7)):
                    pss = [mm_ps.tile([P, TG], f32, tag="mm", name="ps_a"),
                           mm_ps.tile([P, TG], f32, tag="mm", name="ps_b"),
                           y_ps.tile([P, TG], f32, tag="y", name="ps_c"),
                           d_ps.tile([P, TG], f32, tag="d", name="ps_d")]
                    for k in range(KT):
                        for i, m in enumerate(batch):
                            nc.tensor.matmul(
                                pss[i][:],
                                wq_sb[:, k, m * P:(m + 1) * P],
                                xb[:, k],
                                start=(k == 0),
                                stop=(k == KT - 1),
                            )
                    for i, m in enumerate(batch):
                        qkv_post(m, pss[i], i)
            else:
                for mi, m in enumerate((4, 5, 0, 1, 2, 3, 6, 7)):
                    ps = mm_ps.tile([P, TG], f32, tag="mm")
                    for k in range(KT):
                        nc.tensor.matmul(
                            ps[:],
                            wq_sb[:, k, m * P:(m + 1) * P],
                            xb[:, k],
                            start=(k == 0),
                            stop=(k == KT - 1),
                        )
                    qkv_post(m, ps, mi)

            # rope (k quarters first)
            for m in (4, 5, 0, 1, 2, 3):
                xsw = xsws[m]
                t1 = ropet.tile([P, TG], bf16, tag="t1")
                nc.vector.tensor_mul(t1[:], qn[m][:, gsl], cs_sb[:, 0, gsl])
                nc.vector.tensor_mul(xsw[:], xsw[:], cs_sb[:, 1, gsl])
                nc.vector.tensor_add(qn[m][:, gsl], t1[:], xsw[:])

        def emit_attn_outproj(g):
            """Attention + out-projection for token group g."""
            gsl = slice(g * TG, (g + 1) * TG)
            jn = 4 * g + 4
            for qh in range(QH):
                kv = qh // 2
                k_t = qn[4 + kv]
                q_g = qn[qh][:, gsl]
                yp = y_ps.tile([P, TG], f32, tag="y")
                dp = d_ps.tile([P, TG], f32, tag="d")
                pts = []   # (ap, off) pending for the den chain
                for j in range(jn):
                    off = (j - 4 * g) * P if j >= 4 * g else 0
                    diag = j >= 4 * g
                    s = s_ps.tile([P, TG], f32, tag="s")
                    nc.tensor.matmul(
                        s[:, off:],
                        k_t[:, j * P:(j + 1) * P],
                        q_g[:, off:],
                        start=True,
                        stop=not diag,
                        skip_group_check=diag,
                    )
                    if diag:
                        # add -1e30 to the below-diagonal triangle on PE, so
                        # exp maps it to 0 (no cross-engine mask dependency)
                        nc.tensor.matmul(
                            s[:, off:off + P], trineg_sb[:], eye_sb[:],
                            start=False, stop=True,
                            skip_group_check=True,
                        )
                    pt = ptp.tile([P, TG], bf16)
                    nc.scalar.activation(pt[:, off:], s[:, off:], AF.Exp,
                                         scale=SCALE)
                    nc.tensor.matmul(
                        yp[:, off:], vT_sb[:, kv, j], pt[:, off:],
                        start=(j == 0), stop=(j == jn - 1),
                        skip_group_check=True,
                    )
                    # denominator: pair-add full tiles on DVE, chain on PE
                    if off == 0 and j % 2 == 0 and j + 1 < 4 * g:
                        pts.append((pt, -1))    # -1: waiting for partner
                    elif off == 0 and pts and pts[-1][1] == -1:
                        pa = pap.tile([P, TG], bf16)
                        nc.vector.tensor_add(pa[:], pts[-1][0][:], pt[:])
                        pts[-1] = (pa, 0)
                    else:
                        pts.append((pt, off))
                nd = len(pts)
                for i, (pa, off) in enumerate(pts):
                    assert off >= 0
                    nc.tensor.matmul(
                        dp[:, off:], onesm_sb[:], pa[:, off:],
                        start=(i == 0), stop=(i == nd - 1),
                        skip_group_check=True,
                    )
                den = denp.tile([P, TG], f32)
                if qh == QH - 1:
                    # chunk recip+mul per token-tile: the last head gates the
                    # out-projection, so let its first tile finish early
                    for u in range(4):
                        usl = slice(u * P, (u + 1) * P)
                        nc.vector.reciprocal_approx_fast(den[:, usl],
                                                         dp[:, usl])
                        nc.vector.tensor_mul(
                            yT[qh][:, g * TG + u * P: g * TG + (u + 1) * P],
                            yp[:, usl], den[:, usl])
                else:
                    nc.vector.reciprocal_approx_fast(den[:], dp[:])
                    nc.vector.tensor_mul(yT[qh][:, gsl], yp[:], den[:])

            # out-projection for this group
            for tt in range(4 * g, 4 * g + 4):
                ost = ostg.tile([P, C], bf16)
                for og in range(4):
                    op = mm_ps.tile([P, TG], f32, tag="mm", name="op")
                    for h in range(QH):
                        nc.tensor.matmul(
                            op[:], yT[h][:, tt * P:(tt + 1) * P],
                            wp_sb[:, h, og * TG:(og + 1) * TG],
                            start=(h == 0), stop=(h == QH - 1),
                        )
                    if og % 2 == 0:
                        nc.vector.tensor_copy(ost[:, og * TG:(og + 1) * TG], op[:])
                    else:
                        nc.scalar.copy(ost[:, og * TG:(og + 1) * TG], op[:])
                nc.sync.dma_start(out=out_ap[tt * P:(tt + 1) * P, :], in_=ost[:])

        # software pipeline: emit QKV(g+1) BEFORE attention(g) so the
        # scheduler can fill exp-gated attention bubbles with QKV matmuls
        for i in range(G + 1):
            if i < G:
                emit_qkv_rope(i)
            if i >= 1:
                emit_attn_outproj(i - 1)


def build_nc():
    """Build and compile the (single, shared across cores) Bass program."""
    if "nc" in _CACHE:
        return _CACHE["nc"]
    import concourse.mybir as mybir
    import concourse.tile as tile
    from concourse import bacc

    bf16 = mybir.dt.bfloat16

    nc = bacc.Bacc("TRN2", target_bir_lowering=False, debug=False)
    shapes = {
        "x_sw": ((P, G, KT, TG), bf16),
        "wq_sw": ((P, KT, MQ * P), bf16),
        "wp_sw": ((P, QH, C), bf16),
        "cs_sw": ((P, 2, T), bf16),
        "trineg_sw": ((P, P), bf16),
        "eye_sw": ((P, P), bf16),
    }
    t_in = {
        name: nc.dram_tensor(name, shape, dt, kind="ExternalInput").ap()
        for name, (shape, dt) in shapes.items()
    }
    out_ap = nc.dram_tensor("out", (T, C), bf16, kind="ExternalOutput").ap()

    with tile.TileContext(nc) as tc:
        _emit(tc, out_ap, t_in)
    nc.compile()
    _CACHE["nc"] = nc
    return nc


# --------------------------------------------------------------------------
# host-side data preparation
# --------------------------------------------------------------------------

def _swizzle_ktiles(a2d):
    """[R*128, F] -> [128, R, F] picking partition-within-tile as leading."""
    r128, f = a2d.shape
    r = r128 // P
    return np.ascontiguousarray(a2d.reshape(r, P, f).transpose(1, 0, 2))


def host_prep(x, w_attn, w_proj, cos, sin):
    x = np.asarray(x, np.float32)
    w_attn = np.asarray(w_attn, np.float32)
    w_proj = np.asarray(w_proj, np.float32)
    cos = np.asarray(cos, np.float32).reshape(T, HD // 2)
    sin = np.asarray(sin, np.float32).reshape(T, HD // 2)

    # x per batch: (T, C) -> [128, g, k, t]
    x_sws = []
    for b in range(B):
        xb = x[b].reshape(G, TG, KT, P).transpose(3, 0, 2, 1)
        x_sws.append(np.ascontiguousarray(xb).astype(BF16))

    # cos/sin duplicated across both 64-partition halves: [128, 2, T]
    c2 = np.concatenate([cos.T, cos.T], axis=0)     # (128, T)
    s2 = np.concatenate([sin.T, -sin.T], axis=0)    # sign-folded for rope add
    cs_sw = np.stack([c2, s2], axis=1).astype(BF16)  # (128, 2, T)

    col = np.arange(P)[None, :]
    row = np.arange(P)[:, None]
    # M[r,c] = 0 where causal-live (c >= r), -1e30 where masked; the device
    # adds M to the diagonal score block via lhsT = M.T (out[i,j] = lhsT[j,i])
    m_mask = np.where(col >= row, 0.0, -1e30).astype(np.float32)
    trineg_sw = np.ascontiguousarray(m_mask.T).astype(BF16)
    eye_sw = np.eye(P, dtype=np.float32).astype(BF16)

    in_maps = []
    for c in range(N_CORES):
        b, q = divmod(c, 4)
        qrows = w_attn[QH * HD * q: QH * HD * (q + 1)]
        krows = w_attn[C + KVH * HD * q: C + KVH * HD * (q + 1)]
        vrows = w_attn[C + KV_DIM + KVH * HD * q: C + KV_DIM + KVH * HD * (q + 1)]
        w_sel = np.concatenate([qrows, krows, vrows], axis=0)   # (1024, C)
        wq_sw = _swizzle_ktiles(w_sel.T).astype(BF16)           # (128, 16, 1024)

        wp_sel = w_proj[:, QH * HD * q: QH * HD * (q + 1)]      # (C, 512)
        wp_sw = _swizzle_ktiles(np.ascontiguousarray(wp_sel.T)).astype(BF16)

        in_maps.append({
            "x_sw": x_sws[b],
            "wq_sw": np.ascontiguousarray(wq_sw),
            "wp_sw": np.ascontiguousarray(wp_sw),   # (128, 4, 2048)
            "cs_sw": cs_sw,
            "trineg_sw": trineg_sw,
            "eye_sw": eye_sw,
        })
    return in_maps


def run_on_hw(in_maps, trace=False, **kwargs):
    from concourse import bass_utils

    nc = build_nc()
    return bass_utils.run_bass_kernel_spmd(
        nc, in_maps, core_ids=list(range(N_CORES)), trace=trace, **kwargs
    )


def gather(res):
    """Sum the 4 partial outputs per batch -> (B, T, C) float32."""
    out = np.zeros((B, T, C), np.float32)
    for c, r in enumerate(res.results):
        out[c // 4] += r["out"].astype(np.float32)
    return out


def kernel(x, w_attn, w_proj, cos, sin):
    in_maps = host_prep(x, w_attn, w_proj, cos, sin)
    res = run_on_hw(in_maps)
    return gather(res)


# revision 4
# speedup vs baseline: 1.0711x; 1.0001x over previous
"""Trainium2 Bass kernel for nn_CausalSelfAttention (GQA + RoPE + qk-RMSNorm).

Sharding: batch x head-quad over 8 NeuronCores.
  - Core c: batch = c // 4, quad = c % 4.
  - Each core owns 4 of the 16 q heads (4*quad .. 4*quad+3) and the matching
    2 of 8 kv heads (2*quad, 2*quad+1) for ONE batch element.
  - Per core: QKV projection for its 1024 rows of w_attn over its batch's
    2048 tokens, RoPE + qk RMS norm, causal attention, partial output
    projection through its 512 columns of w_proj.
  - Host sums the 4 partial outputs per batch (no on-device collectives).

Fused per-token-group pipeline: for each 512-token group g we run
QKV -> rope/norm -> attention (flash-style, causal-sliced) -> out-proj, so
the tensor engine always has dense matmul work while exp/softmax runs on
the scalar/vector engines.

Matmuls run in bf16 with fp32 PSUM accumulation; softmax/statistics fp32.
Self-contained: hardcodes all shapes from the problem spec.
"""

import math
import numpy as np
import ml_dtypes
from contextlib import ExitStack

# ---- problem constants (hardcoded per spec) ----
B, T, C = 2, 2048, 2048
N_HEAD, N_KV_HEAD, HD = 16, 8, 128
KV_DIM = N_KV_HEAD * HD
EPS = 1.1920929e-07
N_CORES = 8
P = 128
TG = 512                                 # token group (matmul N)
G = T // TG                              # 4 token groups per core
KT = C // P                              # 16 contraction tiles
QH = 4                                   # q heads per core
KVH = 2                                  # kv heads per core
MQ = QH + 2 * KVH                        # 8 row-quarters of the 1024 QKV rows
NJ = T // P                              # 16 k tiles
SCALE = 1.0 / math.sqrt(HD)

BF16 = ml_dtypes.bfloat16

_CACHE = {}


# --------------------------------------------------------------------------
# device program
# --------------------------------------------------------------------------

def _emit(tc, out_ap, t_in):
    import concourse.bass as bass  # noqa: F401
    import concourse.mybir as mybir

    f32 = mybir.dt.float32
    bf16 = mybir.dt.bfloat16
    AF = mybir.ActivationFunctionType
    nc = tc.nc

    x_d = t_in["x_sw"]
    wq_d = t_in["wq_sw"]
    wp_d = t_in["wp_sw"]
    cs_d = t_in["cs_sw"]
    trineg_d = t_in["trineg_sw"]
    eye_d = t_in["eye_sw"]

    with ExitStack() as root:
        const = root.enter_context(tc.tile_pool(name="const", bufs=1))
        xin = root.enter_context(tc.tile_pool(name="xin", bufs=2))
        # fine-grained interleaved staging: QKV(g=0) runs k-outer over the
        # q-head half of wq, so chunk k-tiles of wq/x land just ahead of use.
        # wq on the sync queue, x on the gpsimd queue -- parallel streams.
        wq_sb = const.tile([P, KT, MQ * P], bf16)
        x0_sb = xin.tile([P, KT, TG], bf16, tag="xb")
        for k0 in range(0, KT, 2):
            nc.sync.dma_start(out=wq_sb[:, k0:k0 + 2, 0:4 * P],
                              in_=wq_d[:, k0:k0 + 2, 0:4 * P])
            nc.sync.dma_start(out=x0_sb[:, k0:k0 + 2, :],
                              in_=x_d[:, 0, k0:k0 + 2, :])
        for k0 in range(0, KT, 4):
            nc.sync.dma_start(out=wq_sb[:, k0:k0 + 4, 4 * P:],
                              in_=wq_d[:, k0:k0 + 4, 4 * P:])
        eye_sb = const.tile([P, P], bf16)
        nc.sync.dma_start(out=eye_sb[:], in_=eye_d)
        cs_sb = const.tile([P, 2, T], bf16)
        nc.sync.dma_start(out=cs_sb[:], in_=cs_d)
        trineg_sb = const.tile([P, P], bf16)
        nc.sync.dma_start(out=trineg_sb[:], in_=trineg_d)
        wp_sb = const.tile([P, QH, C], bf16)
        nc.sync.dma_start(out=wp_sb[:], in_=wp_d)
        eps_sb = const.tile([P, 1], f32)
        nc.vector.memset(eps_sb[:], EPS)
        onesm_sb = const.tile([P, P], bf16)
        nc.vector.memset(onesm_sb[:], 1.0)

        big = root.enter_context(tc.tile_pool(name="big", bufs=1))
        # post-rope, post-norm q (4 heads) + k (2 heads), [d, tok] layout
        qn = [big.tile([P, T], bf16, name=f"qn{m}", tag=f"qn{m}")
              for m in range(6)]
        vT_sb = big.tile([P, KVH, NJ, P], bf16, tag="vT")  # [ktok, vh, j, d]
        yT = [big.tile([P, T], bf16, name=f"yT{h}", tag=f"yT{h}")
              for h in range(QH)]

        mm_ps = root.enter_context(tc.tile_pool(name="mmps", bufs=2, space="PSUM"))
        s_ps = root.enter_context(tc.tile_pool(name="sps", bufs=3, space="PSUM"))
        y_ps = root.enter_context(tc.tile_pool(name="yps", bufs=2, space="PSUM"))
        d_ps = root.enter_context(tc.tile_pool(name="dps", bufs=1, space="PSUM"))
        sqp = root.enter_context(tc.tile_pool(name="sq", bufs=3))
        srp = root.enter_context(tc.tile_pool(name="sr", bufs=2))
        ptp = root.enter_context(tc.tile_pool(name="pt", bufs=8))
        pap = root.enter_context(tc.tile_pool(name="pa", bufs=4))
        denp = root.enter_context(tc.tile_pool(name="den", bufs=2))
        vtmp = root.enter_context(tc.tile_pool(name="vtmp", bufs=2))
        xswp = root.enter_context(tc.tile_pool(name="xswp", bufs=6))
        ropet = root.enter_context(tc.tile_pool(name="ropet", bufs=3))
        ostg = root.enter_context(tc.tile_pool(name="ost", bufs=2))

        def emit_qkv_rope(g):
            """QKV projection + norm + rope for token group g."""
            gsl = slice(g * TG, (g + 1) * TG)
            if g == 0:
                xb = x0_sb
            else:
                xb = xin.tile([P, KT, TG], bf16, tag="xb", name="xb")
                nc.sync.dma_start(out=xb[:, 0:8, :], in_=x_d[:, g, 0:8, :])
                nc.sync.dma_start(out=xb[:, 8:16, :], in_=x_d[:, g, 8:16, :])
            xsws = {}

            def qkv_post(m, ps, mi):
                if m < 6:
                    # free the PSUM slot immediately: copy to SBUF first,
                    # then the whole norm chain runs off the SBUF copy, so
                    # an ACT table switch can't back up the matmul pipeline
                    nc.vector.tensor_copy(qn[m][:, gsl], ps[:])
                    # rms-norm: broadcast sum-of-squares via all-ones MM
                    sq = sqp.tile([P, TG], bf16)
                    nc.scalar.activation(sq[:], qn[m][:, gsl], AF.Square)
                    ssq = s_ps.tile([P, TG], f32, tag="s")
                    nc.tensor.matmul(ssq[:], onesm_sb[:], sq[:],
                                     start=True, stop=True)
                    srb = srp.tile([P, TG], f32)
                    nc.scalar.activation(srb[:], ssq[:], AF.Sqrt,
                                         bias=eps_sb[:], scale=1.0 / HD)
                    nc.vector.reciprocal_approx_fast(srb[:], srb[:])
                    nc.vector.tensor_mul(qn[m][:, gsl], qn[m][:, gsl], srb[:])
                    # issue the rope half-swap immediately; consumed after
                    # the m-loop.  Alternate DMA queues to avoid serializing.
                    xsw = xswp.tile([P, TG], bf16, tag="xsw")
                    eng = nc.gpsimd if mi % 2 == 0 else nc.sync
                    eng.dma_start(out=xsw[0:64, :], in_=qn[m][64:128, gsl])
                    eng.dma_start(out=xsw[64:128, :], in_=qn[m][0:64, gsl])
                    xsws[m] = xsw
                else:
                    vh = m - 6
                    vtm = vtmp.tile([P, TG], bf16)
                    nc.vector.tensor_copy(vtm[:], ps[:])
                    for jj in range(4):
                        tp = s_ps.tile([P, P], bf16, tag="s")
                        nc.tensor.transpose(
                            tp[:], vtm[:, jj * P:(jj + 1) * P], eye_sb[:])
                        nc.vector.tensor_copy(vT_sb[:, vh, 4 * g + jj], tp[:])

            if g == 0:
                # k-outer in two 4-quarter batches (4 PSUM banks each) so the
                # PE consumes wq/x chunks as the staged DMAs land
                for batch in ((0, 1, 2, 3), (4, 5, 6, 7)):
                    pss = [mm_ps.tile([P, TG], f32, tag="mm", name="ps_a"),
                           mm_ps.tile([P, TG], f32, tag="mm", name="ps_b"),
                           y_ps.tile([P, TG], f32, tag="y", name="ps_c"),
                           d_ps.tile([P, TG], f32, tag="d", name="ps_d")]
                    for k in range(KT):
                        for i, m in enumerate(batch):
                            nc.tensor.matmul(
                                pss[i][:],
                                wq_sb[:, k, m * P:(m + 1) * P],
                                xb[:, k],
                                start=(k == 0),
                                stop=(k == KT - 1),
                            )
                    for i, m in enumerate(batch):
                        qkv_post(m, pss[i], i)
            else:
                for mi, m in enumerate((4, 5, 0, 1, 2, 3, 6, 7)):
                    ps = mm_ps.tile([P, TG], f32, tag="mm")
                    for k in range(KT):
                        nc.tensor.matmul(
                            ps[:],
                            wq_sb[:, k, m * P:(m + 1) * P],
                            xb[:, k],
                            start=(k == 0),
                            stop=(k == KT - 1),
                        )
                    qkv_post(m, ps, mi)

            return xsws

        def emit_rope(g, xsws):
            """Rope for group g (k quarters first)."""
            gsl = slice(g * TG, (g + 1) * TG)
            for m in (4, 5, 0, 1, 2, 3):
                xsw = xsws[m]
                t1 = ropet.tile([P, TG], bf16, tag="t1")
                nc.vector.tensor_mul(t1[:], qn[m][:, gsl], cs_sb[:, 0, gsl])
                nc.vector.tensor_mul(xsw[:], xsw[:], cs_sb[:, 1, gsl])
                nc.vector.tensor_add(qn[m][:, gsl], t1[:], xsw[:])

        def emit_attn(g):
            """Attention for token group g."""
            gsl = slice(g * TG, (g + 1) * TG)
            jn = 4 * g + 4
            for qh in range(QH):
                kv = qh // 2
                k_t = qn[4 + kv]
                q_g = qn[qh][:, gsl]
                yp = y_ps.tile([P, TG], f32, tag="y")
                dp = d_ps.tile([P, TG], f32, tag="d")
                pts = []   # (ap, off) pending for the den chain
                for j in range(jn):
                    off = (j - 4 * g) * P if j >= 4 * g else 0
                    diag = j >= 4 * g
                    s = s_ps.tile([P, TG], f32, tag="s")
                    nc.tensor.matmul(
                        s[:, off:],
                        k_t[:, j * P:(j + 1) * P],
                        q_g[:, off:],
                        start=True,
                        stop=not diag,
                        skip_group_check=diag,
                    )
                    if diag:
                        # add -1e30 to the below-diagonal triangle on PE, so
                        # exp maps it to 0 (no cross-engine mask dependency)
                        nc.tensor.matmul(
                            s[:, off:off + P], trineg_sb[:], eye_sb[:],
                            start=False, stop=True,
                            skip_group_check=True,
                        )
                    pt = ptp.tile([P, TG], bf16)
                    nc.scalar.activation(pt[:, off:], s[:, off:], AF.Exp,
                                         scale=SCALE)
                    nc.tensor.matmul(
                        yp[:, off:], vT_sb[:, kv, j], pt[:, off:],
                        start=(j == 0), stop=(j == jn - 1),
                        skip_group_check=True,
                    )
                    # denominator: pair-add full tiles on DVE, chain on PE
                    if off == 0 and j % 2 == 0 and j + 1 < 4 * g:
                        pts.append((pt, -1))    # -1: waiting for partner
                    elif off == 0 and pts and pts[-1][1] == -1:
                        pa = pap.tile([P, TG], bf16)
                        nc.vector.tensor_add(pa[:], pts[-1][0][:], pt[:])
                        pts[-1] = (pa, 0)
                    else:
                        pts.append((pt, off))
                nd = len(pts)
                for i, (pa, off) in enumerate(pts):
                    assert off >= 0
                    nc.tensor.matmul(
                        dp[:, off:], onesm_sb[:], pa[:, off:],
                        start=(i == 0), stop=(i == nd - 1),
                        skip_group_check=True,
                    )
                den = denp.tile([P, TG], f32)
                if qh == QH - 1:
                    # chunk recip+mul per token-tile: the last head gates the
                    # out-projection, so let its first tile finish early
                    for u in range(4):
                        usl = slice(u * P, (u + 1) * P)
                        nc.vector.reciprocal_approx_fast(den[:, usl],
                                                         dp[:, usl])
                        nc.vector.tensor_mul(
                            yT[qh][:, g * TG + u * P: g * TG + (u + 1) * P],
                            yp[:, usl], den[:, usl])
                else:
                    nc.vector.reciprocal_approx_fast(den[:], dp[:])
                    nc.vector.tensor_mul(yT[qh][:, gsl], yp[:], den[:])

        def emit_outproj(g):
            """Out-projection for token group g."""
            for tt in range(4 * g, 4 * g + 4):
                ost = ostg.tile([P, C], bf16)
                for og in range(4):
                    op = mm_ps.tile([P, TG], f32, tag="mm", name="op")
                    for h in range(QH):
                        nc.tensor.matmul(
                            op[:], yT[h][:, tt * P:(tt + 1) * P],
                            wp_sb[:, h, og * TG:(og + 1) * TG],
                            start=(h == 0), stop=(h == QH - 1),
                        )
                    if og % 2 == 0:
                        nc.vector.tensor_copy(ost[:, og * TG:(og + 1) * TG], op[:])
                    else:
                        nc.scalar.copy(ost[:, og * TG:(og + 1) * TG], op[:])
                nc.sync.dma_start(out=out_ap[tt * P:(tt + 1) * P, :], in_=ost[:])

        # software pipeline: emit QKV(g+1) BEFORE attention(g) so the
        # scheduler can fill exp-gated attention bubbles with QKV matmuls;
        # attention(g) DVE ops precede rope(g+1) in the vector FIFO (rope
        # has a full iteration of slack, attention gates the out-proj)
        for i in range(G + 1):
            if i < G:
                xsws_i = emit_qkv_rope(i)
            if i >= 1:
                emit_attn(i - 1)
            if i < G:
                emit_rope(i, xsws_i)
            if i >= 1:
                emit_outproj(i - 1)


def build_nc():
    """Build and compile the (single, shared across cores) Bass program."""
    if "nc" in _CACHE:
        return _CACHE["nc"]
    import concourse.mybir as mybir
    import concourse.tile as tile
    from concourse import bacc

    bf16 = mybir.dt.bfloat16

    nc = bacc.Bacc("TRN2", target_bir_lowering=False, debug=False)
    shapes = {
        "x_sw": ((P, G, KT, TG), bf16),
        "wq_sw": ((P, KT, MQ * P), bf16),
        "wp_sw": ((P, QH, C), bf16),
        "cs_sw": ((P, 2, T), bf16),
        "trineg_sw": ((P, P), bf16),
        "eye_sw": ((P, P), bf16),
    }
    t_in = {
        name: nc.dram_tensor(name, shape, dt, kind="ExternalInput").ap()
        for name, (shape, dt) in shapes.items()
    }
    out_ap = nc.dram_tensor("out", (T, C), bf16, kind="ExternalOutput").ap()

    with tile.TileContext(nc) as tc:
        _emit(tc, out_ap, t_in)
    nc.compile()
    _CACHE["nc"] = nc
    return nc


# --------------------------------------------------------------------------
# host-side data preparation
# --------------------------------------------------------------------------

def _swizzle_ktiles(a2d):
    """[R*128, F] -> [128, R, F] picking partition-within-tile as leading."""
    r128, f = a2d.shape
    r = r128 // P
    return np.ascontiguousarray(a2d.reshape(r, P, f).transpose(1, 0, 2))


def host_prep(x, w_attn, w_proj, cos, sin):
    x = np.asarray(x, np.float32)
    w_attn = np.asarray(w_attn, np.float32)
    w_proj = np.asarray(w_proj, np.float32)
    cos = np.asarray(cos, np.float32).reshape(T, HD // 2)
    sin = np.asarray(sin, np.float32).reshape(T, HD // 2)

    # x per batch: (T, C) -> [128, g, k, t]
    x_sws = []
    for b in range(B):
        xb = x[b].reshape(G, TG, KT, P).transpose(3, 0, 2, 1)
        x_sws.append(np.ascontiguousarray(xb).astype(BF16))

    # cos/sin duplicated across both 64-partition halves: [128, 2, T]
    c2 = np.concatenate([cos.T, cos.T], axis=0)     # (128, T)
    s2 = np.concatenate([sin.T, -sin.T], axis=0)    # sign-folded for rope add
    cs_sw = np.stack([c2, s2], axis=1).astype(BF16)  # (128, 2, T)

    col = np.arange(P)[None, :]
    row = np.arange(P)[:, None]
    # M[r,c] = 0 where causal-live (c >= r), -1e30 where masked; the device
    # adds M to the diagonal score block via lhsT = M.T (out[i,j] = lhsT[j,i])
    m_mask = np.where(col >= row, 0.0, -1e30).astype(np.float32)
    trineg_sw = np.ascontiguousarray(m_mask.T).astype(BF16)
    eye_sw = np.eye(P, dtype=np.float32).astype(BF16)

    in_maps = []
    for c in range(N_CORES):
        b, q = divmod(c, 4)
        qrows = w_attn[QH * HD * q: QH * HD * (q + 1)]
        krows = w_attn[C + KVH * HD * q: C + KVH * HD * (q + 1)]
        vrows = w_attn[C + KV_DIM + KVH * HD * q: C + KV_DIM + KVH * HD * (q + 1)]
        w_sel = np.concatenate([qrows, krows, vrows], axis=0)   # (1024, C)
        wq_sw = _swizzle_ktiles(w_sel.T).astype(BF16)           # (128, 16, 1024)

        wp_sel = w_proj[:, QH * HD * q: QH * HD * (q + 1)]      # (C, 512)
        wp_sw = _swizzle_ktiles(np.ascontiguousarray(wp_sel.T)).astype(BF16)

        in_maps.append({
            "x_sw": x_sws[b],
            "wq_sw": np.ascontiguousarray(wq_sw),
            "wp_sw": np.ascontiguousarray(wp_sw),   # (128, 4, 2048)
            "cs_sw": cs_sw,
            "trineg_sw": trineg_sw,
            "eye_sw": eye_sw,
        })
    return in_maps


def run_on_hw(in_maps, trace=False, **kwargs):
    from concourse import bass_utils

    nc = build_nc()
    return bass_utils.run_bass_kernel_spmd(
        nc, in_maps, core_ids=list(range(N_CORES)), trace=trace, **kwargs
    )


def gather(res):
    """Sum the 4 partial outputs per batch -> (B, T, C) float32."""
    out = np.zeros((B, T, C), np.float32)
    for c, r in enumerate(res.results):
        out[c // 4] += r["out"].astype(np.float32)
    return out


def kernel(x, w_attn, w_proj, cos, sin):
    in_maps = host_prep(x, w_attn, w_proj, cos, sin)
    res = run_on_hw(in_maps)
    return gather(res)
